# revision 2
# baseline (speedup 1.0000x reference)
"""MoE (top-2 of 8 experts) Trainium2 kernel, 8-core data-parallel.

Sharding: tokens are split 8 ways with a class-balanced assignment (tokens
sorted by their top-2 expert pair, dealt round-robin) so per-core per-expert
counts are near the global average; capacities (compile-time shapes) come from
host routing with a +2 safety margin.  Host only does layout/dtype prep and
shape planning — the router, top-2 selection, gather, FFN and combine all run
on device.

Per-core device program:
  - fp32 router: logits = x @ Wg + bg, softmax, top-2 -> gates / keep / rank
  - slot position per (token, expert) via prefix-sum matmuls
  - routing metadata (token id, gate) scattered to a DRAM slot table
  - per-expert one-hot matmul gather of the selected tokens (transposed)
  - per-expert FFN in bf16 with fp32 PSUM, exact Gelu, big weight DMAs
  - gated rows scattered to (token, rank) pair slots, pairs combined to out
"""

import os
import sys

for _p in ("/root/.axon_site/_ro/trn_rl_repo", "/opt/trn_rl_repo"):
    if os.path.isdir(_p) and _p not in sys.path:
        sys.path.insert(0, _p)

import numpy as np
import ml_dtypes

import concourse.bass as bass
import concourse.bacc as bacc
import concourse.tile as tile
from concourse import mybir
from concourse.bass_utils import run_bass_kernel_spmd

F32 = mybir.dt.float32
BF16 = mybir.dt.bfloat16
I16 = mybir.dt.int16
I32 = mybir.dt.int32
AF = mybir.ActivationFunctionType
ALU = mybir.AluOpType
BFNP = ml_dtypes.bfloat16

D = 1024      # in_features
H = 4096      # hidden
E = 8         # experts
N_CORES = 8
T = 1024      # tokens per core
NT = T // 128   # token tiles (8)
ND = D // 128   # feature tiles (8)
NH = H // 128   # hidden tiles (32)
NQ = 4          # weight DMA quarters per expert
HTQ = NH // NQ  # hidden tiles per quarter (8)

REPS = 1   # device-side repeat loop (timing only)
SKIP = set()   # crash/timing bisection: {"metascatter", "gather", "scatter_add"}
L1_ACT = AF.Gelu   # CoreSim lacks Gelu; simcheck swaps in Tanh


# ---------------------------------------------------------------- host routing

def route_plan(x, Wg, bg):
    """Host router -> balanced token assignment + capacity plan.

    Returns (perm [M, T] global token ids per core, capp [E], bases [E+1], Sg).
    Only shapes/ordering are derived here; the device recomputes the routing.
    """
    xt = np.asarray(x, np.float32).reshape(-1, D)
    logits = xt @ np.asarray(Wg, np.float32) + np.asarray(bg, np.float32)
    N = logits.shape[0]
    second = np.partition(logits, E - 2, axis=1)[:, E - 2: E - 1]
    sel = logits >= second                      # [N, E] top-2 membership
    top2 = np.argsort(-logits, axis=1, kind="stable")[:, :2]
    e1 = top2.min(1)
    e2 = top2.max(1)
    order = np.argsort(e1 * E + e2, kind="stable")
    # class-sorted round-robin deal: core of order[k] is k % M
    perm = np.stack([order[c::N_CORES] for c in range(N_CORES)])  # [M, T]
    counts = np.stack([sel[perm[c]].sum(0) for c in range(N_CORES)])
    capp = counts.max(0).astype(np.int64) + 2   # +2 host/device divergence margin
    bases = np.concatenate([[0], np.cumsum(capp)]).astype(np.int64)
    S = int(bases[E])
    Sg = ((S + 127) // 128) * 128
    return perm, tuple(int(c) for c in capp), tuple(int(b) for b in bases), Sg


# ---------------------------------------------------------------- device emit

def _emit(nc, tc, io, capp, bases, Sg):
    from contextlib import ExitStack

    SgM = Sg + 128                       # slot table rows (readback headroom)
    meta = nc.dram_tensor("meta", [SgM, 2], F32)

    with ExitStack() as ctx:
        const = ctx.enter_context(tc.tile_pool(name="const", bufs=1))
        xpool = ctx.enter_context(tc.tile_pool(name="xsel", bufs=1))
        rout = ctx.enter_context(tc.tile_pool(name="rout", bufs=1))
        rxt = ctx.enter_context(tc.tile_pool(name="rxt", bufs=2))
        spool = ctx.enter_context(tc.tile_pool(name="smax", bufs=2))
        mpool = ctx.enter_context(tc.tile_pool(name="meta", bufs=4))
        ppool = ctx.enter_context(tc.tile_pool(name="p16", bufs=2))
        xsepool = ctx.enter_context(tc.tile_pool(name="xse", bufs=2))
        cpool = ctx.enter_context(tc.tile_pool(name="comb", bufs=2))
        w1pool = ctx.enter_context(tc.tile_pool(name="w1q", bufs=2))
        w2pool = ctx.enter_context(tc.tile_pool(name="w2q", bufs=2))
        hpool = ctx.enter_context(tc.tile_pool(name="hTs", bufs=2))
        epool = ctx.enter_context(tc.tile_pool(name="eo", bufs=2))
        psA = ctx.enter_context(tc.tile_pool(name="psA", bufs=2, space="PSUM"))
        psB = ctx.enter_context(tc.tile_pool(name="psB", bufs=6, space="PSUM"))

        # ---- constants ----
        ones32 = const.tile([1, 128], F32)
        nc.vector.memset(ones32[:], 1.0)
        ones16 = const.tile([1, 128], BF16)
        nc.vector.memset(ones16[:], 1.0)
        ones128 = const.tile([128, 128], F32)
        nc.vector.memset(ones128[:], 1.0)
        zero4k = const.tile([128, 1024], F32)
        nc.vector.memset(zero4k[:], 0.0)

        lt_i = const.tile([128, 128], I32)            # j - p
        nc.gpsimd.iota(lt_i[:], pattern=[[1, 128]], base=0, channel_multiplier=-1)
        lt_f = const.tile([128, 128], F32)
        nc.vector.tensor_copy(lt_f[:], lt_i[:])
        lstrict = const.tile([128, 128], F32)         # 1 iff p < j
        nc.vector.tensor_scalar(lstrict[:], lt_f[:], 0.0, None, op0=ALU.is_gt)

        # pair-slot ids (+1 so 0 marks an empty slot): rank0 -> 1+tok, rank1 -> 1+T+tok
        tokp_i = const.tile([128, NT], I32)
        nc.gpsimd.iota(tokp_i[:], pattern=[[128, NT]], base=1, channel_multiplier=1)
        tokp_f = const.tile([128, NT], F32)
        nc.vector.tensor_copy(tokp_f[:], tokp_i[:])
        tokpT_i = const.tile([128, NT], I32)
        nc.gpsimd.iota(tokpT_i[:], pattern=[[128, NT]], base=1 + T, channel_multiplier=1)
        tokpT_f = const.tile([128, NT], F32)
        nc.vector.tensor_copy(tokpT_f[:], tokpT_i[:])

        iota_i = const.tile([128, 320], I32)          # column index (one-hot gather)
        nc.gpsimd.iota(iota_i[:], pattern=[[1, 320]], base=0, channel_multiplier=0)
        iota_f = const.tile([128, 320], F32)
        nc.vector.tensor_copy(iota_f[:], iota_i[:])

        wg_t = const.tile([128, ND * E], F32)
        nc.sync.dma_start(
            wg_t[:].rearrange("p (a e) -> p a e", e=E),
            io["Wg"].ap().rearrange("(a p) e -> p a e", p=128),
        )
        bg_t = const.tile([1, E], F32)
        nc.sync.dma_start(bg_t[:], io["bg"].ap())
        b1c_t = const.tile([128, E * NH], F32)
        nc.sync.dma_start(
            b1c_t[:].rearrange("p (e i) -> p e i", i=NH),
            io["b1c"].ap().rearrange("e p i -> p e i"),
        )
        b2r_t = const.tile([1, E * D], BF16)
        nc.sync.dma_start(
            b2r_t[:].rearrange("o (e d) -> o e d", d=D),
            io["b2r"].ap().rearrange("e o d -> o e d"),
        )
        baseb = const.tile([128, NT * E], F32)
        nc.sync.dma_start(baseb[:], io["baseb"].ap())
        capb = const.tile([128, NT * E], F32)
        nc.sync.dma_start(capb[:], io["capb"].ap())

        # ---- zero the slot table (scatter target) & load token-major x ----
        out_d = io["out"].ap()
        nzm = SgM // 128
        nc.sync.dma_start(
            meta.ap().rearrange("(n p) c -> p n c", p=128),
            zero4k[:, : nzm * 2].rearrange("p (n c) -> p n c", c=2),
        )
        xtok = xpool.tile([128, NT * D], BF16)   # [p, tt*D+d] = x[tt*128+p, d]
        nc.sync.dma_start(
            xtok[:].rearrange("p (a d) -> p a d", d=D),
            io["x16"].ap().rearrange("(a p) d -> p a d", p=128),
        )

        # ---- router ----
        g = rout.tile([128, NT * E], F32)
        keepT = rout.tile([128, NT * E], F32)
        eq1 = rout.tile([128, NT * E], F32)
        rk1 = rout.tile([128, NT * E], F32)
        pos = rout.tile([128, NT * E], F32)

        xT32 = io["xT32"].ap()   # [D, T] f32
        for tt in range(NT):
            rx = rxt.tile([128, ND * 128], F32, tag="rx")
            nc.sync.dma_start(
                rx[:].rearrange("p (a t) -> p a t", t=128),
                xT32[:, tt * 128:(tt + 1) * 128].rearrange("(a p) t -> p a t", p=128),
            )
            ps = psA.tile([128, E], F32, tag="psA", name=f"rps_{tt}")
            for dt in range(ND):
                nc.tensor.matmul(
                    ps[:],
                    lhsT=rx[:, dt * 128:(dt + 1) * 128],
                    rhs=wg_t[:, dt * E:(dt + 1) * E],
                    start=(dt == 0),
                    stop=False,
                )
            nc.tensor.matmul(ps[:], lhsT=ones32[:], rhs=bg_t[:], start=False, stop=True)

            ksl = slice(tt * E, (tt + 1) * E)
            logit = spool.tile([128, E], F32, tag="logit")
            nc.scalar.copy(logit[:], ps[:])
            m8 = spool.tile([128, 8], F32, tag="m8")
            nc.vector.max(m8[:], logit[:])
            negm = spool.tile([128, 1], F32, tag="negm")
            nc.vector.tensor_scalar_mul(negm[:], m8[:, 0:1], -1.0)
            p = spool.tile([128, E], F32, tag="p")
            nc.scalar.activation(p[:], logit[:], AF.Exp, bias=negm[:, 0:1])
            s = spool.tile([128, 1], F32, tag="s")
            nc.vector.reduce_sum(s[:], p[:], axis=mybir.AxisListType.X)
            r = spool.tile([128, 1], F32, tag="r")
            nc.vector.reciprocal(r[:], s[:])
            nc.vector.tensor_scalar(
                keepT[:, ksl], logit[:], m8[:, 1:2], None, op0=ALU.is_ge
            )
            nc.vector.tensor_scalar(
                eq1[:, ksl], logit[:], m8[:, 0:1], None, op0=ALU.is_ge
            )
            nc.vector.tensor_tensor(rk1[:, ksl], keepT[:, ksl], eq1[:, ksl], op=ALU.subtract)
            nc.vector.scalar_tensor_tensor(
                out=g[:, ksl],
                in0=p[:],
                scalar=r[:, 0:1],
                in1=keepT[:, ksl],
                op0=ALU.mult,
                op1=ALU.mult,
            )

        # ---- per-expert slot position: exclusive prefix over token tiles ----
        for tt in range(NT):
            ps = psA.tile([128, E], F32, tag="psA", name=f"pps_{tt}")
            for j in range(tt):
                nc.tensor.matmul(
                    ps[:],
                    lhsT=ones128[:],
                    rhs=keepT[:, j * E:(j + 1) * E],
                    start=(j == 0),
                    stop=False,
                )
            nc.tensor.matmul(
                ps[:],
                lhsT=lstrict[:],
                rhs=keepT[:, tt * E:(tt + 1) * E],
                start=(tt == 0),
                stop=True,
            )
            nc.scalar.copy(pos[:, tt * E:(tt + 1) * E], ps[:])

        # ---- scatter (token, gate) into the slot table ----
        posb = rout.tile([128, NT * E], F32)
        nc.vector.tensor_tensor(posb[:], pos[:], baseb[:], op=ALU.add)

        for tt in range(NT if "metavec" not in SKIP else 0):
            ksl = slice(tt * E, (tt + 1) * E)
            vals = mpool.tile([128, 4], F32, tag="vals", name=f"vals_{tt}")
            slots = mpool.tile([128, 2], F32, tag="slots", name=f"slots_{tt}")
            tmp = mpool.tile([128, E], F32, tag="mtmp", name=f"mtmp_{tt}")
            cap_s = mpool.tile([128, 1], F32, tag="mcap", name=f"mcap_{tt}")
            slot_i = mpool.tile([128, 2], I32, tag="slot_i", name=f"sloti_{tt}")
            for r_, mask in ((0, eq1), (1, rk1)):
                nc.vector.tensor_tensor(tmp[:], posb[:, ksl], mask[:, ksl], op=ALU.mult)
                nc.vector.reduce_sum(
                    slots[:, r_:r_ + 1], tmp[:], axis=mybir.AxisListType.X
                )
                nc.vector.tensor_tensor(tmp[:], capb[:, ksl], mask[:, ksl], op=ALU.mult)
                nc.vector.reduce_sum(cap_s[:], tmp[:], axis=mybir.AxisListType.X)
                nc.vector.tensor_tensor(
                    slots[:, r_:r_ + 1], slots[:, r_:r_ + 1], cap_s[:], op=ALU.min
                )
                tsrc = tokp_f if r_ == 0 else tokpT_f
                nc.gpsimd.tensor_copy(vals[:, 2 * r_:2 * r_ + 1], tsrc[:, tt:tt + 1])
                nc.vector.tensor_tensor(tmp[:], g[:, ksl], mask[:, ksl], op=ALU.mult)
                nc.vector.reduce_sum(
                    vals[:, 2 * r_ + 1:2 * r_ + 2], tmp[:], axis=mybir.AxisListType.X
                )
            nc.vector.tensor_copy(slot_i[:], slots[:])
            for r_ in range(2):
                if "metascatter" in SKIP:
                    break
                nc.gpsimd.indirect_dma_start(
                    out=meta.ap(),
                    out_offset=bass.IndirectOffsetOnAxis(
                        ap=slot_i[:, r_:r_ + 1], axis=0
                    ),
                    in_=vals[:, 2 * r_:2 * r_ + 2],
                    in_offset=None,
                    bounds_check=SgM - 1,
                    oob_is_err=False,
                )

        pairs = nc.dram_tensor("pairs", [2 * T, D], BF16)

        # ---- per-expert FFN + gated scatter combine ----
        W1s = io["W1s"].ap()   # [E, NQ, 128, HTQ*ND*128] bf16
        W2s = io["W2s"].ap()   # [E, NQ, 128, HTQ*D] bf16

        for e in range(E if "ffn" not in SKIP else 0):
            C = capp[e]
            base = bases[e]
            JT = (C + 127) // 128

            # one-hot gather: xsel[d, j] = sum_t x[t, d] * [pos[t,e] == j][keep]
            P16 = ppool.tile([128, NT * C], BF16, tag="P16", name=f"P16_{e}")
            for tt in range(NT):
                nc.vector.tensor_scalar(
                    P16[:, tt * C:(tt + 1) * C],
                    iota_f[:, :C],
                    pos[:, tt * E + e: tt * E + e + 1],
                    keepT[:, tt * E + e: tt * E + e + 1],
                    op0=ALU.is_equal,
                    op1=ALU.mult,
                )
            xse = xsepool.tile([128, ND * C], BF16, tag="xse", name=f"xse_{e}")
            for dt in range(ND):
                psg = psA.tile([128, C], F32, tag="psA", name=f"psg_{e}_{dt}")
                for tt in range(NT):
                    nc.tensor.matmul(
                        psg[:],
                        lhsT=xtok[:, tt * D + dt * 128: tt * D + (dt + 1) * 128],
                        rhs=P16[:, tt * C:(tt + 1) * C],
                        start=(tt == 0),
                        stop=(tt == NT - 1),
                    )
                nc.scalar.copy(xse[:, dt * C:(dt + 1) * C], psg[:])

            def xsel_slice(dt, base_, C_):
                return xse[:, dt * C_:(dt + 1) * C_]

            # layer 1: hTs[h, j] = gelu(W1[e].T @ xsel + b1)
            hTs = hpool.tile([128, NH * C], BF16, tag="hTs")
            for q in range(NQ):
                w1q = w1pool.tile([128, HTQ * ND * 128], BF16, tag="w1q")
                nc.sync.dma_start(w1q[:], W1s[e, q])
                for htq in range(HTQ):
                    ht = q * HTQ + htq
                    ps = psA.tile([128, C], F32, tag="psA", name=f"ps1_{e}_{ht}")
                    for dt in range(ND):
                        nc.tensor.matmul(
                            ps[:],
                            lhsT=w1q[:, (htq * ND + dt) * 128:(htq * ND + dt + 1) * 128],
                            rhs=xsel_slice(dt, base, C),
                            start=(dt == 0),
                            stop=(dt == ND - 1),
                        )
                    nc.scalar.activation(
                        hTs[:, ht * C:(ht + 1) * C],
                        ps[:],
                        L1_ACT,
                        bias=b1c_t[:, e * NH + ht: e * NH + ht + 1],
                    )

            if "l2" in SKIP:
                continue
            # layer 2: eo[j, d] = hTs.T @ W2[e] + b2, gated
            pss = [
                [
                    psB.tile([128, 512], F32, tag="ps2", name=f"ps2_{e}_{jt}_{db}")
                    for db in range(2)
                ]
                for jt in range(JT)
            ]
            for q in range(NQ):
                w2q = w2pool.tile([128, HTQ * D], BF16, tag="w2q")
                nc.scalar.dma_start(w2q[:], W2s[e, q])
                for htq in range(HTQ):
                    ht = q * HTQ + htq
                    for jt in range(JT):
                        pj = min(128, C - jt * 128)
                        for db in range(2):
                            nc.tensor.matmul(
                                pss[jt][db][:pj, :],
                                lhsT=hTs[:, ht * C + jt * 128: ht * C + jt * 128 + pj],
                                rhs=w2q[:, htq * D + db * 512: htq * D + db * 512 + 512],
                                start=(ht == 0),
                                stop=False,
                            )

            # gates for this chunk's slots: meta[:, 1] in (n p) layout
            gje0 = mpool.tile([128, JT], F32, tag="gje0", name=f"gje0_{e}")
            nc.sync.dma_start(
                gje0[:].rearrange("p (j c) -> p j c", c=1),
                meta.ap()[base:base + JT * 128, 1:2].rearrange("(j p) c -> p j c", p=128),
            )
            gje = mpool.tile([128, JT], F32, tag="gje", name=f"gje_{e}")
            nc.vector.tensor_copy(gje[:], gje0[:])
            # pair-slot indices for this chunk: meta[:, 0] holds pairslot+1
            psl0 = mpool.tile([128, JT], F32, tag="psl0", name=f"psl0_{e}")
            nc.sync.dma_start(
                psl0[:].rearrange("p (j c) -> p j c", c=1),
                meta.ap()[base:base + JT * 128, 0:1].rearrange("(j p) c -> p j c", p=128),
            )
            emp = mpool.tile([128, JT], F32, tag="emp", name=f"emp_{e}")
            nc.vector.tensor_scalar(emp[:], psl0[:], 0.0, None, op0=ALU.is_equal)
            pslf = mpool.tile([128, JT], F32, tag="pslf", name=f"pslf_{e}")
            nc.vector.scalar_tensor_tensor(
                out=pslf[:], in0=emp[:], scalar=float(4 * T), in1=psl0[:],
                op0=ALU.mult, op1=ALU.add,
            )
            nc.vector.tensor_scalar(pslf[:], pslf[:], 1.0, None, op0=ALU.subtract)
            psl_i = mpool.tile([128, JT], I32, tag="psl_i", name=f"psli_{e}")
            nc.vector.tensor_copy(psl_i[:], pslf[:])

            eo = epool.tile([128, JT * D], BF16, tag="eo")
            for jt in range(JT):
                pj = min(128, C - jt * 128)
                for db in range(2):
                    nc.tensor.matmul(
                        pss[jt][db][:pj, :],
                        lhsT=ones16[:, :pj],
                        rhs=b2r_t[:, e * D + db * 512: e * D + (db + 1) * 512],
                        start=False,
                        stop=True,
                    )
                    nc.vector.tensor_scalar_mul(
                        eo[:pj, jt * D + db * 512: jt * D + (db + 1) * 512],
                        pss[jt][db][:pj, :],
                        gje[:pj, jt: jt + 1],
                    )
                if "scatter" in SKIP:
                    continue
                nc.gpsimd.indirect_dma_start(
                    out=pairs.ap(),
                    out_offset=bass.IndirectOffsetOnAxis(
                        ap=psl_i[:pj, jt:jt + 1], axis=0
                    ),
                    in_=eo[:pj, jt * D:(jt + 1) * D],
                    in_offset=None,
                    bounds_check=2 * T - 1,
                    oob_is_err=False,
                )

        # ---- combine (token, rank) pairs ----
        for mt in range(NT if "ffn" not in SKIP else 0):
            pa = cpool.tile([128, D], BF16, tag="pa", name=f"pa_{mt}")
            nc.sync.dma_start(pa[:], pairs.ap()[mt * 128:(mt + 1) * 128, :])
            pb = cpool.tile([128, D], BF16, tag="pb", name=f"pb_{mt}")
            nc.sync.dma_start(pb[:], pairs.ap()[T + mt * 128: T + (mt + 1) * 128, :])
            po = cpool.tile([128, D], F32, tag="po", name=f"po_{mt}")
            nc.vector.tensor_tensor(po[:], pa[:], pb[:], op=ALU.add)
            nc.scalar.dma_start(out_d[mt * 128:(mt + 1) * 128, :], po[:])


def _build(capp, bases, Sg):
    nc = bacc.Bacc(None, target_bir_lowering=False, debug=False, num_devices=N_CORES)
    io = {
        "xT32": nc.declare_dram_parameter("xT32", [D, T], F32, isOutput=False),
        "x16": nc.declare_dram_parameter("x16", [T, D], BF16, isOutput=False),
        "Wg": nc.declare_dram_parameter("Wg", [D, E], F32, isOutput=False),
        "bg": nc.declare_dram_parameter("bg", [1, E], F32, isOutput=False),
        "W1s": nc.declare_dram_parameter("W1s", [E, NQ, 128, HTQ * ND * 128], BF16, isOutput=False),
        "W2s": nc.declare_dram_parameter("W2s", [E, NQ, 128, HTQ * D], BF16, isOutput=False),
        "b1c": nc.declare_dram_parameter("b1c", [E, 128, NH], F32, isOutput=False),
        "b2r": nc.declare_dram_parameter("b2r", [E, 1, D], BF16, isOutput=False),
        "baseb": nc.declare_dram_parameter("baseb", [128, NT * E], F32, isOutput=False),
        "capb": nc.declare_dram_parameter("capb", [128, NT * E], F32, isOutput=False),
        "out": nc.declare_dram_parameter("out", [T, D], F32, isOutput=True),
    }
    with tile.TileContext(nc) as tc:
        if REPS > 1:
            with tc.For_i(0, REPS, 1):
                _emit(nc, tc, io, capp, bases, Sg)
        else:
            _emit(nc, tc, io, capp, bases, Sg)
    nc.compile()
    return nc


_CACHE = {}


def prep_inputs(x, Wg, bg, W1, b1, W2, b2):
    """Host-side shard + layout/dtype prep. Returns (in_maps, perm, plan)."""
    xt = np.ascontiguousarray(np.asarray(x, dtype=np.float32).reshape(-1, D))
    Wg = np.asarray(Wg, dtype=np.float32)
    bg = np.asarray(bg, dtype=np.float32).reshape(1, E)
    perm, capp, bases, Sg = route_plan(xt, Wg, bg)

    W1b = np.asarray(W1, dtype=np.float32).astype(BFNP)
    W2b = np.asarray(W2, dtype=np.float32).astype(BFNP)
    b1c = np.ascontiguousarray(
        np.asarray(b1, dtype=np.float32).reshape(E, NH, 128).transpose(0, 2, 1)
    )
    b2r = np.asarray(b2, dtype=np.float32).astype(BFNP).reshape(E, 1, D)

    # W1s[e, q, p, (htq*ND + dt)*128 + j] = W1[e, dt*128 + p, (q*HTQ + htq)*128 + j]
    W1s = np.ascontiguousarray(
        W1b.reshape(E, ND, 128, NQ, HTQ, 128)
        .transpose(0, 3, 2, 4, 1, 5)
        .reshape(E, NQ, 128, HTQ * ND * 128)
    )
    # W2s[e, q, p, htq*D + d] = W2[e, (q*HTQ + htq)*128 + p, d]
    W2s = np.ascontiguousarray(
        W2b.reshape(E, NQ, HTQ, 128, D)
        .transpose(0, 1, 3, 2, 4)
        .reshape(E, NQ, 128, HTQ * D)
    )

    basebNT = np.tile(np.asarray(bases[:E], np.float32), (128, NT))
    capbNT = np.tile(
        np.asarray([bases[e] + capp[e] - 1 for e in range(E)], np.float32), (128, NT)
    )

    in_maps = []
    for c in range(N_CORES):
        xs = xt[perm[c]]                     # [T, D]
        xT32 = np.ascontiguousarray(xs.T)    # [D, T]
        in_maps.append(
            {
                "xT32": xT32,
                "x16": xs.astype(BFNP),
                "Wg": Wg,
                "bg": bg,
                "W1s": W1s,
                "W2s": W2s,
                "b1c": b1c,
                "b2r": b2r,
                "baseb": basebNT,
                "capb": capbNT,
            }
        )
    return in_maps, perm, (capp, bases, Sg)


def kernel(x, Wg, bg, W1, b1, W2, b2):
    B_, S_, D_ = x.shape
    in_maps, perm, (capp, bases, Sg) = prep_inputs(x, Wg, bg, W1, b1, W2, b2)
    key = (capp, Sg, REPS)
    if key not in _CACHE:
        _CACHE[key] = _build(capp, bases, Sg)
    nc = _CACHE[key]
    res = run_bass_kernel_spmd(nc, in_maps, list(range(N_CORES)))
    out = np.empty((N_CORES * T, D), np.float32)
    for c in range(N_CORES):
        out[perm[c]] = res.results[c]["out"]
    return out.reshape(B_, S_, D_)


if __name__ == "__main__":
    dat = np.load("/root/problem/_inputs.npz")
    inputs = {k: np.asarray(dat[k]) for k in ("x", "Wg", "bg", "W1", "b1", "W2", "b2")}
    got = kernel(**inputs)
    want = np.asarray(dat["ref"])
    diff = np.abs(got - want)
    scale = np.abs(want).max()
    rel_fro = np.linalg.norm(diff) / np.linalg.norm(want)
    print(f"absmax err: {diff.max():.3e}  absmax/scale: {diff.max() / scale:.3e}  "
          f"rel_fro: {rel_fro:.3e}")
